# revision 10
# baseline (speedup 1.0000x reference)
"""Trainium2 Bass kernel for a small dense transformer (BaseTransformer).

Sharding: pure data-parallel over batch. B=16 -> 2 batch elements per core
across 8 NeuronCores; weights replicated; no collectives.

Device layout choices:
  - Activations kept feature-major in SBUF: x[d_chunk][128, 2*393] fp32.
  - All big matmuls in bf16 (PE runs fp32 at 1/4 rate), fp32 PSUM accum.
  - Block mask handled structurally: the three 128-token agent blocks are
    exactly the mask=1 region, so diagonal 128x128 score blocks use qs/ks
    and all other blocks use q/k.  No mask tensor, no elementwise select.
  - Softmax without max-subtraction (logits are ~unit scale by construction).
  - Attention probs are produced twice:
      [j,i]-layout (transposed) unnormalized exp -> feeds attn @ V directly;
      [i,j]-layout exp with accum_out -> row sums (denominator) + DMA staging,
      normalized in-place by the reciprocal of the row sums.
"""

import os
from contextlib import ExitStack

import numpy as np
import ml_dtypes

import concourse.bass as bass
import concourse.bacc as bacc
import concourse.mybir as mybir
import concourse.tile as tile
from concourse.bass_utils import run_bass_kernel_spmd

BF = mybir.dt.bfloat16
F32 = mybir.dt.float32
AF = mybir.ActivationFunctionType
ALU = mybir.AluOpType
AX = mybir.AxisListType

NCORES = 8
B = 16
BPC = B // NCORES          # batch elements per core
LOOK = 128
NAGENT = 3
NC_, NB_, NT_ = 9, 12, 8
NIN = NC_ + NB_ + NT_      # 29
D, H, DH, FC, DEPTH = 512, 8, 64, 2048, 4
T = NAGENT * LOOK + NC_    # 393
TT = BPC * T               # 786 tokens held per core (2 batch elements)
NPRED = 20
DC = D // 128              # 4 feature chunks
FCC = FC // 128            # 16
SCALE = DH ** -0.5

TOK_TILES = [(0, 128), (128, 256), (256, 384), (384, 393)]

LAST_EXEC_NS = None
_BUILT = None


def _bf(x):
    return np.asarray(x).astype(ml_dtypes.bfloat16)


def _sin_pe_np(positions, dim):
    pos = np.asarray(positions, np.float32)[:, None]
    i = np.arange(0, dim, 2, dtype=np.float32)
    div = np.exp(-np.log(10000.0) * i / dim)
    ang = pos * div
    pe = np.zeros((pos.shape[0], dim), np.float32)
    pe[:, 0::2] = np.sin(ang)
    pe[:, 1::2] = np.cos(ang)
    return pe


def _score_segments(a):
    """Column segments for a score tile whose 128-token block index is `a`:
    [(c0, c1, use_special)].  Diagonal agent blocks use the qs/ks scores."""
    t0, t1 = TOK_TILES[a]
    if a < 3:
        segs = []
        if t0 > 0:
            segs.append((0, t0, False))
        segs.append((t0, t1, True))
        if t1 < T:
            segs.append((t1, T, False))
        return segs
    return [(0, T, False)]


def _build_graph():
    """Build and compile the Bacc graph (same SPMD program on all cores)."""
    nc = bacc.Bacc("TRN2", target_bir_lowering=False, debug=False,
                   num_devices=NCORES)

    dt = nc.dram_tensor
    # ---- inputs (per-core shards / replicated weights) ----
    xin_d = dt("xinT", [BPC, NIN, LOOK], BF, kind="ExternalInput")
    spec_d = dt("spec_bf", [1, BPC * NC_], BF, kind="ExternalInput")
    wc_d = dt("wc", [NC_, D], BF, kind="ExternalInput")
    wb_d = dt("wb", [NB_, D], BF, kind="ExternalInput")
    wt_d = dt("wt", [NT_, D], BF, kind="ExternalInput")
    specwt_d = dt("specWT", [D, NC_], F32, kind="ExternalInput")
    specbt_d = dt("specBT", [D, NC_], F32, kind="ExternalInput")
    pe_d = dt("peplus", [D, NAGENT * LOOK], F32, kind="ExternalInput")
    qkv_d = dt("qkvW", [DEPTH, D, 3 * D], BF, kind="ExternalInput")
    qks_d = dt("qksW", [DEPTH, D, 2 * D], BF, kind="ExternalInput")
    outw_d = dt("outW", [DEPTH, D, D], BF, kind="ExternalInput")
    ff1_d = dt("ff1W", [DEPTH, D, FC], BF, kind="ExternalInput")
    ff2_d = dt("ff2W", [DEPTH, FC, D], BF, kind="ExternalInput")
    ln1g_d = dt("ln1gT", [DEPTH, 128, DC], F32, kind="ExternalInput")
    ln1b_d = dt("ln1bT", [DEPTH, 128, DC], F32, kind="ExternalInput")
    ln2g_d = dt("ln2gT", [DEPTH, 128, DC], F32, kind="ExternalInput")
    ln2b_d = dt("ln2bT", [DEPTH, 128, DC], F32, kind="ExternalInput")
    outb_d = dt("outbT", [DEPTH, 128, DC], F32, kind="ExternalInput")
    ffb1_d = dt("ffb1T", [DEPTH, 128, FCC], F32, kind="ExternalInput")
    ffb2_d = dt("ffb2T", [DEPTH, 128, DC], F32, kind="ExternalInput")
    geng_d = dt("gengT", [128, DC], F32, kind="ExternalInput")
    genbl_d = dt("genblT", [128, DC], F32, kind="ExternalInput")
    genw_d = dt("genW", [D, NPRED], BF, kind="ExternalInput")
    genb_d = dt("genb2", [BPC, NPRED], F32, kind="ExternalInput")
    onesbf_d = dt("ones_bf", [1, 128], BF, kind="ExternalInput")
    onesc_d = dt("ones_c", [128, 1], BF, kind="ExternalInput")   # value 1/512
    i128_d = dt("i128_bf", [128, 128], BF, kind="ExternalInput")

    # ---- outputs ----
    out_d = dt("out", [BPC, NPRED], F32, kind="ExternalOutput")
    attn_d = dt("attn", [DEPTH, BPC, H, T, T], F32, kind="ExternalOutput")

    with tile.TileContext(nc) as tc, ExitStack() as ctx:
        pool1 = ctx.enter_context(tc.tile_pool(name="pool1", bufs=1))
        pool2 = ctx.enter_context(tc.tile_pool(name="pool2", bufs=2))
        pool4 = ctx.enter_context(tc.tile_pool(name="pool4", bufs=4))
        pool8 = ctx.enter_context(tc.tile_pool(name="pool8", bufs=8))
        pool16 = ctx.enter_context(tc.tile_pool(name="pool16", bufs=16))
        psA = ctx.enter_context(tc.tile_pool(name="psA", bufs=3, space="PSUM"))
        psB = ctx.enter_context(tc.tile_pool(name="psB", bufs=4, space="PSUM"))

        _uid = [0]

        def tl(pool, shape, dtype, tag, bufs=None):
            _uid[0] += 1
            return pool.tile(shape, dtype, tag=tag, bufs=bufs,
                             name=f"{tag}_{_uid[0]}")

        # ---------- long-lived constants ----------
        ones_bf = tl(pool1, [1, 128], BF, "ones_bf")
        nc.sync.dma_start(ones_bf[:], onesbf_d[:])
        ones_c = tl(pool1, [128, 1], BF, "ones_c")
        nc.sync.dma_start(ones_c[:], onesc_d[:])
        i128 = tl(pool1, [128, 128], BF, "i128")
        nc.sync.dma_start(i128[:], i128_d[:])

        # ---------- persistent residual stream ----------
        x_sb = [tl(pool4, [128, TT], F32, "x") for _ in range(DC)]

        # ---------- input embedding (early-released pool) ----------
        with tc.tile_pool(name="pembed", bufs=1) as pem:
            def tle(shape, dtype, tag):
                _uid[0] += 1
                return pem.tile(shape, dtype, tag=tag,
                                name=f"{tag}_{_uid[0]}")

            wc_sb = tle([NC_, D], BF, "wc")
            nc.sync.dma_start(wc_sb[:], wc_d[:])
            wb_sb = tle([NB_, D], BF, "wb")
            nc.sync.dma_start(wb_sb[:], wb_d[:])
            wt_sb = tle([NT_, D], BF, "wt")
            nc.sync.dma_start(wt_sb[:], wt_d[:])
            xinc = tle([NC_, BPC * LOOK], BF, "xinc")
            xinb = tle([NB_, BPC * LOOK], BF, "xinb")
            xint = tle([NT_, BPC * LOOK], BF, "xint")
            for b in range(BPC):
                nc.sync.dma_start(xinc[:, b * LOOK:(b + 1) * LOOK],
                                  xin_d[b, 0:NC_])
                nc.sync.dma_start(xinb[:, b * LOOK:(b + 1) * LOOK],
                                  xin_d[b, NC_:NC_ + NB_])
                nc.sync.dma_start(xint[:, b * LOOK:(b + 1) * LOOK],
                                  xin_d[b, NC_ + NB_:NIN])
            spec_sb = tle([1, BPC * NC_], BF, "spec")
            nc.sync.dma_start(spec_sb[:], spec_d[:])

            pe_sb, swt_sb, sbt_sb = [], [], []
            pe_r = pe_d.ap().rearrange("(c p) f -> c p f", p=128)
            swt_r = specwt_d.ap().rearrange("(c p) f -> c p f", p=128)
            sbt_r = specbt_d.ap().rearrange("(c p) f -> c p f", p=128)
            for c in range(DC):
                _uid[0] += 1
                t_ = pem.tile([128, NAGENT * LOOK], F32, tag=f"pe{c}",
                              name=f"pe_{_uid[0]}")
                nc.sync.dma_start(t_[:], pe_r[c])
                pe_sb.append(t_)
                _uid[0] += 1
                t_ = pem.tile([128, NC_], F32, tag=f"swt{c}",
                              name=f"swt_{_uid[0]}")
                nc.sync.dma_start(t_[:], swt_r[c])
                swt_sb.append(t_)
                _uid[0] += 1
                t_ = pem.tile([128, NC_], F32, tag=f"sbt{c}",
                              name=f"sbt_{_uid[0]}")
                nc.sync.dma_start(t_[:], sbt_r[c])
                sbt_sb.append(t_)

            blocks = [(wc_sb, xinc), (wb_sb, xinb), (wt_sb, xint)]
            for b in range(BPC):
                for m in range(DC):
                    for blk, (w_sb, xi_sb) in enumerate(blocks):
                        ps = tl(psA, [128, 128], F32, "mm")
                        nc.tensor.matmul(
                            ps[:], w_sb[:, m * 128:(m + 1) * 128],
                            xi_sb[:, b * LOOK:(b + 1) * LOOK],
                            start=True, stop=True)
                        nc.vector.tensor_add(
                            x_sb[m][:, b * T + blk * 128:
                                    b * T + (blk + 1) * 128],
                            ps[:], pe_sb[m][:, blk * 128:(blk + 1) * 128])
                # spec tokens (last 9): x = specW.T * spec_bcast + specB.T
                psb = tl(psA, [128, NC_], F32, "mm")
                nc.tensor.matmul(psb[:], ones_bf[:, 0:128],
                                 spec_sb[:, b * NC_:(b + 1) * NC_],
                                 start=True, stop=True)
                for m in range(DC):
                    tmp = tle([128, NC_], F32, f"sptmp{b}{m}")
                    nc.vector.tensor_mul(tmp[:], swt_sb[m][:], psb[:])
                    nc.vector.tensor_add(
                        x_sb[m][:, b * T + 3 * 128: b * T + 3 * 128 + NC_],
                        tmp[:], sbt_sb[m][:])

        # ---------- transformer layers ----------
        qkv_r = qkv_d.ap().rearrange("l (c p) f -> l c p f", p=128)
        qks_r = qks_d.ap().rearrange("l (c p) f -> l c p f", p=128)
        outw_r = outw_d.ap().rearrange("l (c p) f -> l c p f", p=128)
        ff1_r = ff1_d.ap().rearrange("l (c p) f -> l c p f", p=128)
        ff2_r = ff2_d.ap().rearrange("l (c p) f -> l c p f", p=128)

        def layernorm(g_sb, b_sb):
            """LN over features of x_sb (fp32, feature-major).

            g_sb/b_sb: [128, DC] fp32, column c = chunk c.
            Returns h: list of DC [128, TT] bf16 tiles."""
            xb = [tl(pool4, [128, TT], BF, "xb") for _ in range(DC)]
            for c in range(DC):
                nc.vector.tensor_copy(xb[c][:], x_sb[c][:])
            h = [tl(pool4, [128, TT], BF, "h") for _ in range(DC)]
            for b in range(BPC):
                cols = slice(b * T, (b + 1) * T)
                mean_ps = tl(psB, [1, T], F32, "sc")
                for c in range(DC):
                    nc.tensor.matmul(mean_ps[:], ones_c[:], xb[c][:, cols],
                                     start=(c == 0), stop=(c == DC - 1))
                msq_ps = tl(psB, [1, T], F32, "sc")
                for c in range(DC):
                    x2 = tl(pool2, [128, T], BF, "x2")
                    nc.vector.tensor_mul(x2[:], xb[c][:, cols], xb[c][:, cols])
                    nc.tensor.matmul(msq_ps[:], ones_c[:], x2[:],
                                     start=(c == 0), stop=(c == DC - 1))
                mu_f = tl(pool2, [1, T], F32, "muf")
                nc.scalar.copy(mu_f[:], mean_ps[:])
                st = tl(pool2, [1, T], F32, "lnstat")
                nc.vector.tensor_mul(st[:], mu_f[:], mu_f[:])        # mu^2
                nc.vector.tensor_sub(st[:], msq_ps[:], st[:])        # var
                nc.vector.tensor_scalar_add(st[:], st[:], 1e-5)
                nc.vector.reciprocal(st[:], st[:])                   # 1/var
                rs_bf = tl(pool2, [1, T], BF, "rsbf")
                nc.scalar.activation(rs_bf[:], st[:], AF.Sqrt)       # rsigma
                mu_bf = tl(pool2, [1, T], BF, "mubf")
                nc.vector.tensor_copy(mu_bf[:], mu_f[:])
                mu_bc = tl(psB, [128, T], F32, "sc")
                nc.tensor.matmul(mu_bc[:], ones_bf[:], mu_bf[:],
                                 start=True, stop=True)
                rs_bc = tl(psB, [128, T], F32, "sc")
                nc.tensor.matmul(rs_bc[:], ones_bf[:], rs_bf[:],
                                 start=True, stop=True)
                for c in range(DC):
                    t1 = tl(pool4, [128, T], BF, "lnt1")
                    nc.vector.tensor_sub(t1[:], xb[c][:, cols], mu_bc[:])
                    t2 = tl(pool4, [128, T], BF, "lnt2")
                    nc.vector.tensor_mul(t2[:], t1[:], rs_bc[:])
                    nc.scalar.activation(h[c][:, cols], t2[:], AF.Identity,
                                         bias=b_sb[:, c:c + 1],
                                         scale=g_sb[:, c:c + 1])
            return h

        for l in range(DEPTH):
            # ---- layer weights ----
            wqkv = [tl(pool4, [128, 3 * D], BF, "wqkv") for _ in range(DC)]
            wqks = [tl(pool4, [128, 2 * D], BF, "wqks") for _ in range(DC)]
            wout = [tl(pool4, [128, D], BF, "wout") for _ in range(DC)]
            wff1 = [tl(pool4, [128, FC], BF, "wff1") for _ in range(DC)]
            wff2 = [tl(pool16, [128, D], BF, "wff2") for _ in range(FCC)]
            for c in range(DC):
                nc.sync.dma_start(wqkv[c][:], qkv_r[l, c])
                nc.sync.dma_start(wqks[c][:], qks_r[l, c])
                nc.sync.dma_start(wout[c][:], outw_r[l, c])
                nc.sync.dma_start(wff1[c][:], ff1_r[l, c])
            for c in range(FCC):
                nc.sync.dma_start(wff2[c][:], ff2_r[l, c])
            l1g = tl(pool2, [128, DC], F32, "l1g")
            nc.sync.dma_start(l1g[:], ln1g_d[l])
            l1b = tl(pool2, [128, DC], F32, "l1b")
            nc.sync.dma_start(l1b[:], ln1b_d[l])
            l2g = tl(pool2, [128, DC], F32, "l2g")
            nc.sync.dma_start(l2g[:], ln2g_d[l])
            l2b = tl(pool2, [128, DC], F32, "l2b")
            nc.sync.dma_start(l2b[:], ln2b_d[l])
            outb = tl(pool2, [128, DC], F32, "outb")
            nc.sync.dma_start(outb[:], outb_d[l])
            ffb1 = tl(pool2, [128, FCC], F32, "ffb1")
            nc.sync.dma_start(ffb1[:], ffb1_d[l])
            ffb2 = tl(pool2, [128, DC], F32, "ffb2")
            nc.sync.dma_start(ffb2[:], ffb2_d[l])

            # ---- LN1 ----
            h1 = layernorm(l1g, l1b)

            for b in range(BPC):
                cols = slice(b * T, (b + 1) * T)
                cols384 = slice(b * T, b * T + 384)
                # ---- q, k, qs, ks for this batch element ----
                q_sb = [tl(pool4, [128, T], BF, "q") for _ in range(DC)]
                k_sb = [tl(pool4, [128, T], BF, "k") for _ in range(DC)]
                qs_sb = [tl(pool4, [128, T], BF, "qs") for _ in range(DC)]
                ks_sb = [tl(pool4, [128, T], BF, "ks") for _ in range(DC)]
                for ft in range(2 * DC):   # q then k feature tiles
                    dst = q_sb[ft] if ft < DC else k_sb[ft - DC]
                    ps = tl(psA, [128, T], F32, "mm")
                    for c in range(DC):
                        nc.tensor.matmul(ps[:],
                                         wqkv[c][:, ft * 128:(ft + 1) * 128],
                                         h1[c][:, cols],
                                         start=(c == 0), stop=(c == DC - 1))
                    if ft < DC:
                        nc.scalar.mul(dst[:], ps[:], SCALE)
                    else:
                        nc.vector.tensor_copy(dst[:], ps[:])
                for ft in range(2 * DC):   # qs then ks (tokens 0:384 only)
                    dst = qs_sb[ft] if ft < DC else ks_sb[ft - DC]
                    ps = tl(psA, [128, T], F32, "mm")
                    for c in range(DC):
                        nc.tensor.matmul(ps[:, 0:384],
                                         wqks[c][:, ft * 128:(ft + 1) * 128],
                                         h1[c][:, cols384],
                                         start=(c == 0), stop=(c == DC - 1))
                    if ft < DC:
                        nc.scalar.mul(dst[:, 0:384], ps[:, 0:384], SCALE)
                    else:
                        nc.vector.tensor_copy(dst[:, 0:384], ps[:, 0:384])
                # ---- v (token-major) ----
                v_sb = []
                for (t0, t1) in TOK_TILES:
                    pt = t1 - t0
                    ps = tl(psA, [128, D], F32, "mm")
                    for c in range(DC):
                        nc.tensor.matmul(ps[0:pt, :],
                                         h1[c][:, b * T + t0: b * T + t1],
                                         wqkv[c][:, 2 * D:3 * D],
                                         start=(c == 0), stop=(c == DC - 1))
                    vt = tl(pool8, [128, D], BF, "v")
                    nc.vector.tensor_copy(vt[0:pt, :], ps[0:pt, :])
                    v_sb.append(vt)

                # ---- attention ----
                o_n = {}
                for p in range(H // 2):
                    rt_sbs = []
                    aut = {}
                    for hh in range(2):
                        hidx = 2 * p + hh
                        hc = hidx // 2
                        ro = 64 * (hidx % 2)
                        qa, ka = q_sb[hc], k_sb[hc]
                        qsa, ksa = qs_sb[hc], ks_sb[hc]
                        # scores_T [j, i] -> exp -> attnU_T (for attn@V)
                        for a, (j0, j1) in enumerate(TOK_TILES):
                            pj = j1 - j0
                            st = tl(psB, [128, T], F32, "sc")
                            for (s0, s1, diag) in _score_segments(a):
                                lh = (ksa if diag else ka)[ro:ro + 64, j0:j1]
                                rh = (qsa if diag else qa)[ro:ro + 64, s0:s1]
                                nc.tensor.matmul(st[0:pj, s0:s1], lh, rh,
                                                 start=True, stop=True)
                            au = tl(pool16, [128, T], BF, "aut")
                            nc.scalar.activation(au[0:pj, :], st[0:pj, :],
                                                 AF.Exp)
                            aut[(hh, a)] = au
                        # scores [i, j] -> exp (+row sums) -> staging fp32
                        den = tl(pool2, [128, 4], F32, "den")
                        nc.gpsimd.memset(den[:], 1.0)
                        ae_tiles = []
                        for ti, (t0, t1) in enumerate(TOK_TILES):
                            pi = t1 - t0
                            sp = tl(psB, [128, T], F32, "sc")
                            for (s0, s1, diag) in _score_segments(ti):
                                lh = (qsa if diag else qa)[ro:ro + 64, t0:t1]
                                rh = (ksa if diag else ka)[ro:ro + 64, s0:s1]
                                nc.tensor.matmul(sp[0:pi, s0:s1], lh, rh,
                                                 start=True, stop=True)
                            ae = tl(pool8, [128, T], F32, "ae")
                            nc.scalar.activation(
                                ae[0:pi, :], sp[0:pi, :], AF.Exp,
                                accum_out=den[0:pi, ti:ti + 1])
                            ae_tiles.append(ae)
                        rec = tl(pool2, [128, 4], F32, "rec")
                        nc.vector.reciprocal(rec[:], den[:])
                        rb = tl(pool2, [128, 4], BF, "rb")
                        nc.vector.tensor_copy(rb[:], rec[:])
                        # normalize staging in-place + DMA attn maps out
                        for ti, (t0, t1) in enumerate(TOK_TILES):
                            pi = t1 - t0
                            ae = ae_tiles[ti]
                            nc.vector.tensor_scalar_mul(
                                ae[0:pi, :], ae[0:pi, :],
                                rec[0:pi, ti:ti + 1])
                            nc.sync.dma_start(
                                attn_d[l, b, hidx, t0:t1, :], ae[0:pi, :])
                        # recip transposed to free-major [1, T]
                        rt_ps = tl(psB, [1, T], F32, "sc")
                        for ti, (t0, t1) in enumerate(TOK_TILES):
                            pi = t1 - t0
                            nc.tensor.matmul(rt_ps[0:1, t0:t1],
                                             rb[0:pi, ti:ti + 1],
                                             i128[0:pi, 0:pi],
                                             start=True, stop=True)
                        rt_sb = tl(pool4, [1, T], BF, "rtsb")
                        nc.scalar.copy(rt_sb[:], rt_ps[:])
                        rt_sbs.append(rt_sb)
                    # ---- attn @ V for the pair (col-tiled) ----
                    ou = tl(psA, [128, T], F32, "ou", bufs=1)
                    for hh in range(2):
                        hidx = 2 * p + hh
                        ro = 64 * hh
                        for a, (j0, j1) in enumerate(TOK_TILES):
                            pj = j1 - j0
                            nc.tensor.matmul(
                                ou[ro:ro + 64, :],
                                v_sb[a][0:pj, hidx * DH:(hidx + 1) * DH],
                                aut[(hh, a)][0:pj, :],
                                start=(a == 0), stop=(a == 3),
                                tile_position=(0, ro),
                                skip_group_check=True)
                    # broadcast per-head recip rows and normalize o
                    bc = tl(psB, [128, T], F32, "sc")
                    nc.tensor.matmul(bc[0:64, :], ones_bf[:, 0:64],
                                     rt_sbs[0][:], start=True, stop=True,
                                     tile_position=(0, 0),
                                     skip_group_check=True)
                    nc.tensor.matmul(bc[64:128, :], ones_bf[:, 0:64],
                                     rt_sbs[1][:], start=True, stop=True,
                                     tile_position=(0, 64),
                                     skip_group_check=True)
                    bcs = tl(pool4, [128, T], BF, "bcs")
                    nc.vector.tensor_copy(bcs[:], bc[:])
                    on = tl(pool4, [128, T], BF, "on")
                    nc.vector.tensor_mul(on[:], ou[:], bcs[:])
                    o_n[p] = on

                # ---- output projection + residual ----
                for m in range(DC):
                    ps = tl(psA, [128, T], F32, "mm")
                    for c in range(DC):
                        nc.tensor.matmul(ps[:],
                                         wout[c][:, m * 128:(m + 1) * 128],
                                         o_n[c][:],
                                         start=(c == 0), stop=(c == DC - 1))
                    nc.vector.scalar_tensor_tensor(
                        x_sb[m][:, cols], ps[:], outb[:, m:m + 1],
                        x_sb[m][:, cols], op0=ALU.add, op1=ALU.add)

            # ---- LN2 + FFN ----
            h2 = layernorm(l2g, l2b)
            for b in range(BPC):
                cols = slice(b * T, (b + 1) * T)
                hg = [tl(pool16, [128, T], BF, "hg") for _ in range(FCC)]
                for m in range(FCC):
                    ps = tl(psA, [128, T], F32, "mm")
                    for c in range(DC):
                        nc.tensor.matmul(ps[:],
                                         wff1[c][:, m * 128:(m + 1) * 128],
                                         h2[c][:, cols],
                                         start=(c == 0), stop=(c == DC - 1))
                    if os.environ.get("BASS_SIM_GELU"):
                        # CoreSim has no Gelu; approximate x*sigmoid(1.702x)
                        xb_ = tl(pool2, [128, T], F32, "gx")
                        nc.scalar.activation(xb_[:], ps[:], AF.Identity,
                                             bias=ffb1[:, m:m + 1])
                        sg = tl(pool2, [128, T], F32, "gs")
                        nc.scalar.activation(sg[:], xb_[:], AF.Sigmoid,
                                             scale=1.702)
                        nc.vector.tensor_mul(hg[m][:], xb_[:], sg[:])
                    else:
                        nc.scalar.activation(hg[m][:], ps[:], AF.Gelu,
                                             bias=ffb1[:, m:m + 1])
                for m in range(DC):
                    ps = tl(psA, [128, T], F32, "mm")
                    for c in range(FCC):
                        nc.tensor.matmul(ps[:],
                                         wff2[c][:, m * 128:(m + 1) * 128],
                                         hg[c][:],
                                         start=(c == 0), stop=(c == FCC - 1))
                    nc.vector.scalar_tensor_tensor(
                        x_sb[m][:, cols], ps[:], ffb2[:, m:m + 1],
                        x_sb[m][:, cols], op0=ALU.add, op1=ALU.add)

        # ---------- final head ----------
        gg_sb = tl(pool1, [128, DC], F32, "geng")
        nc.sync.dma_start(gg_sb[:], geng_d[:])
        gbl_sb = tl(pool1, [128, DC], F32, "genbl")
        nc.sync.dma_start(gbl_sb[:], genbl_d[:])
        genb_sb = tl(pool1, [BPC, NPRED], F32, "genb")
        nc.sync.dma_start(genb_sb[:], genb_d[:])
        genw_sb = []
        genw_r = genw_d.ap().rearrange("(c p) f -> c p f", p=128)
        for c in range(DC):
            t_ = tl(pool4, [128, NPRED], BF, "genw")
            nc.sync.dma_start(t_[:], genw_r[c])
            genw_sb.append(t_)

        pooled = tl(pool1, [128, DC * BPC], F32, "pooled")
        for c in range(DC):
            for b in range(BPC):
                nc.vector.tensor_reduce(
                    pooled[:, c * BPC + b: c * BPC + b + 1],
                    x_sb[c][:, b * T:(b + 1) * T], axis=AX.X, op=ALU.add)
        nc.vector.tensor_scalar_mul(pooled[:], pooled[:], 1.0 / T)
        pool_bf = tl(pool1, [128, DC * BPC], BF, "poolbf")
        nc.vector.tensor_copy(pool_bf[:], pooled[:])
        p2 = tl(pool1, [128, DC * BPC], BF, "pool2bf")
        nc.vector.tensor_mul(p2[:], pool_bf[:], pool_bf[:])
        mean_ps = tl(psB, [1, BPC], F32, "sc")
        for c in range(DC):
            nc.tensor.matmul(mean_ps[:], ones_c[:],
                             pool_bf[:, c * BPC:(c + 1) * BPC],
                             start=(c == 0), stop=(c == DC - 1))
        msq_ps = tl(psB, [1, BPC], F32, "sc")
        for c in range(DC):
            nc.tensor.matmul(msq_ps[:], ones_c[:],
                             p2[:, c * BPC:(c + 1) * BPC],
                             start=(c == 0), stop=(c == DC - 1))
        mu_f = tl(pool1, [1, BPC], F32, "hmuf")
        nc.scalar.copy(mu_f[:], mean_ps[:])
        st = tl(pool1, [1, BPC], F32, "hstat")
        nc.vector.tensor_mul(st[:], mu_f[:], mu_f[:])
        nc.vector.tensor_sub(st[:], msq_ps[:], st[:])
        nc.vector.tensor_scalar_add(st[:], st[:], 1e-5)
        nc.vector.reciprocal(st[:], st[:])
        rs_bf = tl(pool1, [1, BPC], BF, "hrsbf")
        nc.scalar.activation(rs_bf[:], st[:], AF.Sqrt)
        mu_bf = tl(pool1, [1, BPC], BF, "hmubf")
        nc.vector.tensor_copy(mu_bf[:], mu_f[:])
        mu_bc = tl(psB, [128, BPC], F32, "sc")
        nc.tensor.matmul(mu_bc[:], ones_bf[:], mu_bf[:], start=True, stop=True)
        rs_bc = tl(psB, [128, BPC], F32, "sc")
        nc.tensor.matmul(rs_bc[:], ones_bf[:], rs_bf[:], start=True, stop=True)
        pn = tl(pool1, [128, DC * BPC], BF, "pn")
        for c in range(DC):
            t1 = tl(pool2, [128, BPC], BF, "ht1")
            nc.vector.tensor_sub(t1[:], pool_bf[:, c * BPC:(c + 1) * BPC],
                                 mu_bc[:])
            t2 = tl(pool2, [128, BPC], BF, "ht2")
            nc.vector.tensor_mul(t2[:], t1[:], rs_bc[:])
            nc.scalar.activation(pn[:, c * BPC:(c + 1) * BPC], t2[:],
                                 AF.Identity, bias=gbl_sb[:, c:c + 1],
                                 scale=gg_sb[:, c:c + 1])
        # out[b, :] = pooledN[:, b] @ genW + genb
        outp = tl(psA, [BPC, NPRED], F32, "mm")
        for c in range(DC):
            nc.tensor.matmul(outp[:], pn[:, c * BPC:(c + 1) * BPC],
                             genw_sb[c][:], start=(c == 0), stop=(c == DC - 1))
        out_sb = tl(pool1, [BPC, NPRED], F32, "outsb")
        nc.vector.tensor_add(out_sb[:], outp[:], genb_sb[:])
        nc.sync.dma_start(out_d[:], out_sb[:])

    nc.compile()
    return nc


def _prepare_maps(inputs):
    """Host-side input prep: dtype casts + layout transposes + constants."""
    f32 = np.float32
    pe_a = _sin_pe_np(np.repeat(np.arange(NAGENT), LOOK), D)
    pe_t = _sin_pe_np(np.tile(np.arange(LOOK), NAGENT), D)
    pe = pe_a + pe_t                                   # [384, D]
    bias_blocks = np.concatenate([
        np.broadcast_to(np.asarray(inputs["bc"], f32), (LOOK, D)),
        np.broadcast_to(np.asarray(inputs["bb"], f32), (LOOK, D)),
        np.broadcast_to(np.asarray(inputs["bt"], f32), (LOOK, D)),
    ], axis=0)
    pe_plus = np.ascontiguousarray((pe + bias_blocks).T.astype(f32))  # [D,384]

    def perlayer_T(v, cols):   # [DEPTH, cols*128] -> [DEPTH, 128, cols]
        a = np.asarray(v, f32).reshape(DEPTH, cols, 128)
        return np.ascontiguousarray(a.transpose(0, 2, 1))

    common = {
        "wc": _bf(inputs["Wc"]), "wb": _bf(inputs["Wb"]),
        "wt": _bf(inputs["Wt"]),
        "specWT": np.ascontiguousarray(np.asarray(inputs["spec_W"], f32).T),
        "specBT": np.ascontiguousarray(np.asarray(inputs["spec_b"], f32).T),
        "peplus": pe_plus,
        "qkvW": _bf(inputs["qkv_W"]), "qksW": _bf(inputs["qks_W"]),
        "outW": _bf(inputs["out_W"]),
        "ff1W": _bf(inputs["ff_W1"]), "ff2W": _bf(inputs["ff_W2"]),
        "ln1gT": perlayer_T(inputs["ln1_g"], DC),
        "ln1bT": perlayer_T(inputs["ln1_b"], DC),
        "ln2gT": perlayer_T(inputs["ln2_g"], DC),
        "ln2bT": perlayer_T(inputs["ln2_b"], DC),
        "outbT": perlayer_T(inputs["out_b"], DC),
        "ffb1T": perlayer_T(inputs["ff_b1"], FCC),
        "ffb2T": perlayer_T(inputs["ff_b2"], DC),
        "gengT": np.ascontiguousarray(
            np.asarray(inputs["gen_g"], f32).reshape(DC, 128).T),
        "genblT": np.ascontiguousarray(
            np.asarray(inputs["gen_b_ln"], f32).reshape(DC, 128).T),
        "genW": _bf(inputs["gen_W"]),
        "genb2": np.broadcast_to(
            np.asarray(inputs["gen_b"], f32), (BPC, NPRED)).copy(),
        "ones_bf": _bf(np.ones((1, 128))),
        "ones_c": _bf(np.full((128, 1), 1.0 / D)),
        "i128_bf": _bf(np.eye(128)),
    }
    x_in = np.asarray(inputs["x_in"], f32)     # [B, 128, 29]
    spec = np.asarray(inputs["spec"], f32)     # [B, 9]
    in_maps = []
    for c in range(NCORES):
        xs = x_in[c * BPC:(c + 1) * BPC]       # [2, 128, 29]
        m = dict(common)
        m["xinT"] = _bf(np.ascontiguousarray(xs.transpose(0, 2, 1)))
        m["spec_bf"] = _bf(spec[c * BPC:(c + 1) * BPC].reshape(1, BPC * NC_))
        in_maps.append(m)
    return in_maps


def kernel(**inputs):
    global _BUILT, LAST_EXEC_NS
    if _BUILT is None:
        _BUILT = _build_graph()
    nc = _BUILT
    in_maps = _prepare_maps(inputs)
    trace = bool(int(os.environ.get("BENCH_TRACE", "0")))
    res = run_bass_kernel_spmd(nc, in_maps, core_ids=list(range(NCORES)),
                               trace=trace)
    LAST_EXEC_NS = res.exec_time_ns
    outs = np.zeros((B, NPRED), np.float32)
    attn = np.zeros((DEPTH, B, H, T, T), np.float32)
    for c in range(NCORES):
        r = res.results[c]
        outs[c * BPC:(c + 1) * BPC] = r["out"]
        attn[:, c * BPC:(c + 1) * BPC] = r["attn"]
    return outs, attn.reshape(DEPTH * B, H, T, T)
